# revision 1
# baseline (speedup 1.0000x reference)
"""DiffusionGraphConv (3-hop symmetric-normalized diffusion + linear) on 8 TRN2 cores.

Math (reference):
    deg  = segment_sum(1, dst); norm = clip(deg,1)^-0.5
    h_0  = feat
    h_k  = norm * segment_sum(norm[src] * h_{k-1}[src] -> dst)
    out  = concat(h_0..h_3) @ W.T + b

Reformulation (all linear; norms folded into per-edge/per-node scalings):
    g_k = norm * h_k
    s_k = segment_sum(g_{k-1}[src] -> dst)   # pure gather + segment-sum
    h_k = norm * s_k ; out = feat @ W0.T + sum_k h_k @ Wk.T + b
    hop 1 gathers raw feat with norm[src] folded into the one-hot matrix,
    so g_0 is never materialized and needs no exchange.

Distribution: nodes (and their edges, by dst) sharded across 8 cores.
Per hop, each core DMA-gathers source rows, does the segment-sum as
one-hot matmuls on the TensorEngine (128-edge blocks into 128-node PSUM
windows; norm[dst] is folded into the one-hot matrix), and AllGathers its
updated node shard for the next hop. Each shard is split into an A region
(windows 0-24) and B region (windows 25-48) exchanged by two separate
AllGathers: A launches mid-hop and overlaps the rest of the hop's compute;
next hop's A-region gathers only wait on A. Gather indices are region-
relative so they fit int16. The final linear is data-parallel over node
shards with replicated W.
"""

import math
import sys

sys.path.insert(0, "/opt/trn_rl_repo")

import numpy as np

import concourse.bacc as bacc
import concourse.mybir as mybir
import concourse.tile as tile
import concourse.tile_sem_assignment as _tsa
from concourse.bass_utils import run_bass_kernel_spmd

# Problem constants (hardcoded per the harness contract).
N = 50000
E = 800000
D = 64
HOPS = 3
NCORES = 8
SHARD = N // NCORES          # 6250 nodes per core
NWIN = (SHARD + 127) // 128  # 49 windows of 128 nodes
SHARD_PAD = NWIN * 128       # 6272
WA = 25                      # windows in region A
WB = NWIN - WA               # windows in region B
ROWSA = WA * 128             # 3200 rows per shard in region A
ROWSB = WB * 128             # 3072 rows per shard in region B
REGA = NCORES * ROWSA        # 25600 (int16-safe region sizes)
REGB = NCORES * ROWSB        # 24576
NCELLS = NWIN * 2            # (window, region) cells per core

F32 = mybir.dt.float32
I16 = mybir.dt.int16


def _set_problem(n, e):
    """Recompute derived constants for a different problem size (testing)."""
    global N, E, SHARD, NWIN, SHARD_PAD, WA, WB, ROWSA, ROWSB, REGA, REGB, NCELLS
    N, E = n, e
    SHARD = N // NCORES
    NWIN = (SHARD + 127) // 128
    SHARD_PAD = NWIN * 128
    WA = (NWIN + 1) // 2
    WB = NWIN - WA
    ROWSA = WA * 128
    ROWSB = WB * 128
    REGA = NCORES * ROWSA
    REGB = NCORES * ROWSB
    NCELLS = NWIN * 2
    assert REGA < 32768 and REGB < 32768

# Timing aid: repeat the computation REPS times inside one NEFF so
# (T(R) - T(1)) / (R - 1) cancels host/dispatch overhead. Leave at 1.
REPS = 1
# Timing-experiment knobs (leave defaults for the graded kernel).
NO_COLLECTIVE = False   # replace AllGathers with local DMA (wrong results)
GATHERS_ONLY = False    # hops do only the dma_gather stream (wrong results)
NO_GATHERS = False      # hops skip dma_gather (compute on stale tiles)
NQUEUES = 4             # SWDGE queues; gathers round-robin across them
GCH = 768               # max slots per dma_gather (SWDGE ring is ~1024 descs)


# The ucode locks each SWDGE completion semaphore to one queue, but Tile's
# pass-1 lane assigner round-robins Pool DMA instructions across all 8 DMASW
# lanes queue-unaware. Partition the lanes per queue instead: queue q owns
# lanes {q*L .. q*L+L-1}, chosen from the instruction's queue_num.
_orig_assign_tick = _tsa.TileClockTick._assign_tick


def _queue_aware_assign_tick(self, inst):
    q = getattr(inst, "queue_num", None)
    if (
        q is not None
        and inst.engine == mybir.EngineType.Pool
        and isinstance(inst, _tsa.DMAInst)
        and not isinstance(inst, _tsa.bass_isa.UserSyncedRemoteDMADescs)
    ):
        lanes_per_q = max(1, self.swdge_sem_count // max(1, NQUEUES))
        if not hasattr(self, "_q_lane_ctr"):
            self._q_lane_ctr = {}
        c = self._q_lane_ctr.get(q, 0)
        self._q_lane_ctr[q] = c + 1
        self.next_sw_dma_idx = (q % self.swdge_sem_count) * lanes_per_q % (
            self.swdge_sem_count
        ) + (c % lanes_per_q)
    return _orig_assign_tick(self, inst)


_tsa.TileClockTick._assign_tick = _queue_aware_assign_tick


def _preprocess(src, dst):
    """Build per-core gather/segment metadata from the edge list."""
    src = np.asarray(src).astype(np.int64)
    dst = np.asarray(dst).astype(np.int64)

    deg = np.bincount(dst, minlength=N).astype(np.float32)
    norm = np.clip(deg, 1.0, None) ** -0.5

    core = dst // SHARD
    dst_loc = dst - core * SHARD
    win = dst_loc >> 7
    dst_in_win = (dst_loc & 127).astype(np.float32)

    # region-relative gather rows for the source endpoint
    src_r = src // SHARD
    src_i = src % SHARD
    in_b = (src_i >= ROWSA).astype(np.int64)
    rel = np.where(in_b == 0, src_r * ROWSA + src_i, src_r * ROWSB + (src_i - ROWSA))

    cell = (core * NCELLS + win * 2 + in_b).astype(np.int64)
    order = np.lexsort((rel, cell))
    cell_s = cell[order]
    rel_s = rel[order]
    dw_s = dst_in_win[order]
    nsrc_s = norm[src[order]]

    counts = np.bincount(cell_s, minlength=NCORES * NCELLS)
    starts = np.zeros(NCORES * NCELLS + 1, np.int64)
    np.cumsum(counts, out=starts[1:])
    pos = np.arange(E) - starts[cell_s]

    counts_pc = counts.reshape(NCORES, NCELLS)
    nvalid = np.maximum(counts_pc.max(axis=0), 1).astype(np.int64)  # [NCELLS]
    slots_h = int(math.ceil(nvalid.max() / 128.0) * 128)
    tot = NCELLS * slots_h

    idx_slots = np.full((NCORES, NCELLS, slots_h), -1, np.int16)
    dloc_slots = np.full((NCORES, NCELLS, slots_h), -1.0, np.float32)
    nsrc_slots = np.zeros((NCORES, NCELLS, slots_h), np.float32)
    c_s = cell_s // NCELLS
    l_s = cell_s % NCELLS
    idx_slots[c_s, l_s, pos] = rel_s.astype(np.int16)
    dloc_slots[c_s, l_s, pos] = dw_s
    nsrc_slots[c_s, l_s, pos] = nsrc_s
    # fake fill [count, nvalid): idx 0 (valid row), dloc -1 (zero one-hot row)
    grid = np.arange(slots_h)[None, None, :]
    fake = (grid >= counts_pc[:, :, None]) & (grid < nvalid[None, :, None])
    idx_slots[fake] = 0

    idx_tiles, dloc_tiles, nsrc_tiles = [], [], []
    for c in range(NCORES):
        it = idx_slots[c].reshape(tot // 16, 16).T  # slot j at [j%16, j//16]
        idx_tiles.append(np.tile(it, (8, 1)).copy())
        dloc_tiles.append(dloc_slots[c].reshape(tot // 128, 128).T.copy())
        nsrc_tiles.append(nsrc_slots[c].reshape(tot // 128, 128).T.copy())

    return norm, idx_tiles, dloc_tiles, nsrc_tiles, nvalid, slots_h


def _regionize(x):
    """[N or N-padded, D] node-ordered -> [REGA+REGB, D] region layout."""
    out = np.zeros((REGA + REGB, x.shape[1]), x.dtype)
    for r in range(NCORES):
        sh = x[r * SHARD : (r + 1) * SHARD]
        out[r * ROWSA : r * ROWSA + ROWSA] = sh[:ROWSA]
        nb = SHARD - ROWSA
        out[REGA + r * ROWSB : REGA + r * ROWSB + nb] = sh[ROWSA:]
    return out


def _build(slots_h, nvalid):
    """Build the 8-core SPMD Bass program (same program on every core)."""
    nc = bacc.Bacc(
        "TRN2",
        target_bir_lowering=False,
        debug=False,
        num_devices=NCORES,
        num_swdge_queues=NQUEUES,
    )

    tot = NCELLS * slots_h

    feat_full_p = nc.declare_dram_parameter(
        "feat_full", [REGA + REGB, D], F32, isOutput=False
    )
    featT_p = nc.declare_dram_parameter("featT", [D, SHARD_PAD], F32, isOutput=False)
    idx_p = nc.declare_dram_parameter("idx", [128, tot // 16], I16, isOutput=False)
    dloc_p = nc.declare_dram_parameter("dloc", [128, tot // 128], F32, isOutput=False)
    nsrc_p = nc.declare_dram_parameter("nsrc", [128, tot // 128], F32, isOutput=False)
    nrow_p = nc.declare_dram_parameter("nrow", [128, SHARD_PAD], F32, isOutput=False)
    ncol_p = nc.declare_dram_parameter("ncol", [128, NWIN], F32, isOutput=False)
    wt_p = nc.declare_dram_parameter("wt", [D, 4 * D], F32, isOutput=False)
    bias_p = nc.declare_dram_parameter("bias", [128, D], F32, isOutput=False)
    iota_p = nc.declare_dram_parameter("iota", [128, 128], F32, isOutput=False)
    ident_p = nc.declare_dram_parameter("ident", [128, 128], F32, isOutput=False)
    out_p = nc.declare_dram_parameter("out", [SHARD_PAD, D], F32, isOutput=True)

    with tile.TileContext(nc) as tc:
        with (
            tc.tile_pool(name="meta", bufs=1) as meta,
            tc.tile_pool(name="gpool", bufs=3) as gpool,
            tc.tile_pool(name="spool", bufs=4) as spool,
            tc.tile_pool(name="work", bufs=3) as work,
            tc.tile_pool(name="hstore", bufs=1) as hstore_pool,
            tc.tile_pool(name="ps", bufs=4, space="PSUM") as ps_pool,
            tc.tile_pool(name="pso", bufs=2, space="PSUM") as pso_pool,
            tc.tile_pool(name="pst", bufs=2, space="PSUM") as pst_pool,
            tc.tile_pool(name="dram", bufs=1, space="DRAM") as dram,
        ):
            # ---- metadata preload (resident in SBUF); idx first so hop-1
            # gathers can start as soon as it lands ----
            idx_sb = meta.tile([128, tot // 16], I16)
            nc.sync.dma_start(idx_sb[:], idx_p[:])
            dloc_sb = meta.tile([128, tot // 128], F32)
            nc.sync.dma_start(dloc_sb[:], dloc_p[:])
            nsrc_sb = meta.tile([128, tot // 128], F32)
            nc.sync.dma_start(nsrc_sb[:], nsrc_p[:])
            nrow_sb = meta.tile([128, SHARD_PAD], F32)
            nc.sync.dma_start(nrow_sb[:], nrow_p[:])
            ncol_sb = meta.tile([128, NWIN], F32)
            nc.sync.dma_start(ncol_sb[:], ncol_p[:])
            wt_sb = meta.tile([D, 4 * D], F32)
            nc.sync.dma_start(wt_sb[:], wt_p[:])
            bias_sb = meta.tile([128, D], F32)
            nc.sync.dma_start(bias_sb[:], bias_p[:])
            iota_sb = meta.tile([128, 128], F32)
            nc.sync.dma_start(iota_sb[:], iota_p[:])
            ident_sb = meta.tile([128, 128], F32)
            nc.sync.dma_start(ident_sb[:], ident_p[:])
            featT_sb = meta.tile([D, SHARD_PAD], F32)
            nc.sync.dma_start(featT_sb[:], featT_p[:])

            # h_k tiles for k=1,2 kept for the final linear
            hstore = hstore_pool.tile([128, (HOPS - 1) * NWIN * D], F32)

            # explicit multi-buffered gather tiles, memset once so skipped
            # (-1-padded) slots always hold finite values
            NGBUF = 3
            nblk = 2 * (slots_h // 128)
            gtiles = []
            for i in range(NGBUF):
                gt = gpool.tile([128, nblk, D], F32, tag="G", name=f"G{i}")
                nc.vector.memset(gt[:], 0.0)
                gtiles.append(gt)

            # per-hop-boundary A/B exchange buffers (hop 1 and 2 outputs)
            g_inA = [dram.tile([ROWSA, D], F32, name=f"g_inA{k}") for k in range(2)]
            g_inB = [dram.tile([ROWSB, D], F32, name=f"g_inB{k}") for k in range(2)]
            g_fullA = [dram.tile([REGA, D], F32, name=f"g_fullA{k}") for k in range(2)]
            g_fullB = [dram.tile([REGB, D], F32, name=f"g_fullB{k}") for k in range(2)]

            for _rep in range(REPS):
                _phases(
                    nc, slots_h, nvalid, feat_full_p, out_p,
                    g_inA, g_inB, g_fullA, g_fullB, gtiles,
                    idx_sb, dloc_sb, nsrc_sb, nrow_sb, ncol_sb, wt_sb, bias_sb,
                    iota_sb, ident_sb, featT_sb, hstore,
                    work, spool, ps_pool, pso_pool, pst_pool,
                )
    nc.compile()
    return nc


AG_ENGINE = "gpsimd"  # engine issuing the collective doorbell


def _ag(nc, g_in, g_full):
    if NO_COLLECTIVE:
        nc.sync.dma_start(g_full[0 : g_in.shape[0], :], g_in[:, :])
    else:
        eng = getattr(nc, AG_ENGINE)
        eng.collective_compute(
            "AllGather",
            mybir.AluOpType.bypass,
            replica_groups=[list(range(NCORES))],
            ins=[g_in.opt()],
            outs=[g_full.opt()],
        )


def _phases(
    nc, slots_h, nvalid, feat_full_p, out_p,
    g_inA, g_inB, g_fullA, g_fullB, gtiles,
    idx_sb, dloc_sb, nsrc_sb, nrow_sb, ncol_sb, wt_sb, bias_sb,
    iota_sb, ident_sb, featT_sb, hstore,
    work, spool, ps_pool, pso_pool, pst_pool,
):
    NGBUF = len(gtiles)
    nblk = 2 * (slots_h // 128)
    for k in range(1, HOPS + 1):
        if k == 1:
            srcA = feat_full_p[0:REGA, :]
            srcB = feat_full_p[REGA : REGA + REGB, :]
        else:
            srcA = g_fullA[k - 2][:, :]
            srcB = g_fullB[k - 2][:, :]
        for w in range(NWIN):
            gt = gtiles[w % NGBUF]
            if not NO_GATHERS:
                for h in range(2):
                    cellid = w * 2 + h
                    col0 = cellid * (slots_h // 16)
                    src_ap = srcA if h == 0 else srcB
                    for j0 in range(0, slots_h, GCH):
                        nidx = min(GCH, slots_h - j0)
                        vld = min(max(int(nvalid[cellid]) - j0, 0), nidx)
                        if vld == 0:
                            continue
                        b0 = h * (slots_h // 128) + j0 // 128
                        nc.gpsimd.dma_gather(
                            gt[:, b0 : b0 + nidx // 128, :],
                            src_ap,
                            idx_sb[:, col0 + j0 // 16 : col0 + (j0 + nidx) // 16],
                            nidx,
                            vld,
                            D,
                            elem_step=D,
                            queue_num=(w * 2 + h) % NQUEUES,
                        )
            if GATHERS_ONLY:
                continue
            ps = ps_pool.tile([128, D], F32)
            for b in range(nblk):
                s_t = spool.tile([128, 128], F32, tag="S")
                blkcol = w * nblk + b
                # S[e,v] = (dst_loc[e]==v) * norm[node v of window w]
                nc.vector.scalar_tensor_tensor(
                    s_t[:],
                    iota_sb[:],
                    dloc_sb[:, blkcol : blkcol + 1],
                    nrow_sb[:, w * 128 : (w + 1) * 128],
                    mybir.AluOpType.is_equal,
                    mybir.AluOpType.mult,
                )
                if k == 1:
                    # hop 1 gathers raw feat: fold norm[src] in as well
                    # (on the otherwise-idle Scalar engine)
                    s2 = spool.tile([128, 128], F32, tag="S2")
                    nc.scalar.activation(
                        s2[:],
                        s_t[:],
                        mybir.ActivationFunctionType.Copy,
                        scale=nsrc_sb[:, blkcol : blkcol + 1],
                    )
                    s_t = s2
                nc.tensor.matmul(
                    ps[:],
                    s_t[:],
                    gt[:, b, :],
                    start=(b == 0),
                    stop=(b == nblk - 1),
                )
            # ps now holds h_k for window w (norm[dst] folded via nrow)
            if k < HOPS:
                hslice = hstore[
                    :, ((k - 1) * NWIN + w) * D : ((k - 1) * NWIN + w + 1) * D
                ]
                nc.vector.tensor_copy(hslice, ps[:])
                gsb = work.tile([128, D], F32, tag="gsb")
                nc.vector.tensor_scalar_mul(gsb[:], ps[:], ncol_sb[:, w : w + 1])
                if w < WA:
                    nc.sync.dma_start(
                        g_inA[k - 1][w * 128 : (w + 1) * 128, :], gsb[:]
                    )
                else:
                    nc.sync.dma_start(
                        g_inB[k - 1][(w - WA) * 128 : (w - WA + 1) * 128, :], gsb[:]
                    )
                if w == WA - 1:
                    # region A complete: exchange it while B still computes
                    _ag(nc, g_inA[k - 1], g_fullA[k - 1])
            else:
                # final linear for window w
                po = pso_pool.tile([128, D], F32)
                nc.tensor.matmul(
                    po[:],
                    featT_sb[:, w * 128 : (w + 1) * 128],
                    wt_sb[:, 0:D],
                    start=True,
                    stop=False,
                )
                for kk in range(1, HOPS + 1):
                    if kk < HOPS:
                        hsrc = hstore[
                            :, ((kk - 1) * NWIN + w) * D : ((kk - 1) * NWIN + w + 1) * D
                        ]
                    else:
                        h3 = work.tile([128, D], F32, tag="h3")
                        nc.vector.tensor_copy(h3[:], ps[:])
                        hsrc = h3[:]
                    pt = pst_pool.tile([D, 128], F32)
                    nc.tensor.matmul(pt[:], hsrc, ident_sb[:], is_transpose=True)
                    hT = work.tile([D, 128], F32, tag="hT")
                    nc.vector.tensor_copy(hT[:], pt[:])
                    nc.tensor.matmul(
                        po[:],
                        hT[:],
                        wt_sb[:, kk * D : (kk + 1) * D],
                        start=False,
                        stop=(kk == HOPS),
                    )
                osb = work.tile([128, D], F32, tag="osb")
                nc.vector.tensor_add(osb[:], po[:], bias_sb[:])
                nc.sync.dma_start(out_p[w * 128 : (w + 1) * 128, :], osb[:])
        if k < HOPS and not GATHERS_ONLY:
            _ag(nc, g_inB[k - 1], g_fullB[k - 1])
        elif GATHERS_ONLY and k < HOPS:
            # keep hop ordering honest for timing runs
            _ag(nc, g_inA[k - 1], g_fullA[k - 1])
            _ag(nc, g_inB[k - 1], g_fullB[k - 1])


def _make_in_maps(feat, src, dst, W, b):
    feat = np.ascontiguousarray(np.asarray(feat), dtype=np.float32)
    W = np.ascontiguousarray(np.asarray(W), dtype=np.float32)
    b = np.ascontiguousarray(np.asarray(b), dtype=np.float32)

    norm, idx_tiles, dloc_tiles, nsrc_tiles, nvalid, slots_h = _preprocess(src, dst)

    feat_full = _regionize(feat)
    wt = np.concatenate(
        [W[:, k * D : (k + 1) * D].T for k in range(HOPS + 1)], axis=1
    ).copy()
    bias = np.tile(b[None, :], (128, 1)).copy()
    iota = np.tile(np.arange(128, dtype=np.float32)[None, :], (128, 1)).copy()
    ident = np.eye(128, dtype=np.float32)

    in_maps = []
    for c in range(NCORES):
        fs = np.zeros((SHARD_PAD, D), np.float32)
        fs[:SHARD] = feat[c * SHARD : (c + 1) * SHARD]
        ns = np.zeros(SHARD_PAD, np.float32)
        ns[:SHARD] = norm[c * SHARD : (c + 1) * SHARD]
        in_maps.append(
            {
                "feat_full": feat_full,
                "featT": fs.T.copy(),
                "idx": idx_tiles[c],
                "dloc": dloc_tiles[c],
                "nsrc": nsrc_tiles[c],
                "nrow": np.tile(ns[None, :], (128, 1)).copy(),
                "ncol": ns.reshape(NWIN, 128).T.copy(),
                "wt": wt,
                "bias": bias,
                "iota": iota,
                "ident": ident,
            }
        )
    return in_maps, nvalid, slots_h


def _run(feat, src, dst, W, b, trace=False):
    in_maps, nvalid, slots_h = _make_in_maps(feat, src, dst, W, b)
    nc = _build(slots_h, nvalid)
    res = run_bass_kernel_spmd(nc, in_maps, list(range(NCORES)), trace=trace)
    out = np.concatenate(
        [res.results[c]["out"][:SHARD] for c in range(NCORES)], axis=0
    )
    return out, res


def kernel(feat, src, dst, W, b):
    out, _ = _run(feat, src, dst, W, b, trace=False)
    return out


def kernel_traced(feat, src, dst, W, b):
    return _run(feat, src, dst, W, b, trace=True)

